# revision 10
# baseline (speedup 1.0000x reference)
"""Trainium2 Bass kernel for a dense transformer block (B=4, T=2048, C=1024,
16 heads, dff=4096, causal attention, erf-GELU FFN, LN + residuals).

Sharding over 8 NeuronCores: core c handles sequence b = c//2 and head-group
g = c%2 (8 of 16 heads).  Attention is computed head-sharded for all 2048
tokens; a pairwise ReduceScatter re-shards by token so each core runs proj +
LN2 + FFN on its 1024-token half.  All activations live feature-on-partition
("transposed" layout) so no on-chip transposes are needed:

  x^T --LN1--> h^T --W-stationary mm--> q^T,k^T ; --h-stationary mm--> v
  s^T[k,q] = k^T.T @ q^T   (2 heads packed in the 128-row PE array, K=64)
  p^T = exp(s^T/8) masked causally; att@v uses v (with a ones column -> also
  yields the softmax denominator l); y^T normalized via a K=1 broadcast
  matmul of 1/l, then exchanged (ReduceScatter of a zero-masked buffer: the
  row region for the other core's head-group is zeroed by a shipped per-core
  mask, so the program is rank-invariant), then proj/FFN as plain
  W^T-stationary matmuls.  Residual adds stay fp32 end to end.
"""

import os
import numpy as np
import ml_dtypes
from contextlib import ExitStack

import json

import concourse.bass as bass
import concourse.tile as tile
import concourse.bass2jax as bass2jax
from concourse import mybir
from concourse.bass_utils import run_bass_kernel_spmd
from concourse.bass_utils import compile_bir_kernel as _orig_compile_bir_kernel


def _legalize_waits(bir_bytes):
    """walrus's per-instruction sync-wait slots are tight (a DVE op with two
    attached waits fails codegen with 'Too many sync wait commands').  Hoist
    all-but-one wait of every instruction into standalone EventSemaphore
    instructions on the same engine queue, which is semantically identical
    (queues block at the hoisted wait before dispatching the instruction)."""
    j = json.loads(bir_bytes)
    uid = [0]
    for fn in j.get("functions", []):
        for blk in fn.get("blocks", []):
            insts = blk.get("instructions", [])
            out = []
            for ins in insts:
                si = ins.get("sync_info")
                if (si and len(si.get("on_wait", [])) > 1
                        and ins.get("opcode") != "EventSemaphore"):
                    waits = si["on_wait"]
                    for w in waits[:-1]:
                        uid[0] += 1
                        out.append({
                            "debug": ins.get("debug", 0),
                            "engine": ins["engine"],
                            "ins": [], "outs": [],
                            "name": f"hoistw-{uid[0]}",
                            "opcode": "EventSemaphore",
                            "sync_info": {"on_update": [], "on_wait": [w]},
                        })
                    si["on_wait"] = [waits[-1]]
                out.append(ins)
            blk["instructions"] = out
    return json.dumps(j).encode()


def _patched_compile_bir_kernel(bir_bytes, *args, **kwargs):
    return _orig_compile_bir_kernel(_legalize_waits(bir_bytes), *args, **kwargs)


bass2jax.compile_bir_kernel = _patched_compile_bir_kernel

BF16 = mybir.dt.bfloat16
F32 = mybir.dt.float32
AF = mybir.ActivationFunctionType
ALU = mybir.AluOpType

P = 128
B, T, C = 4, 2048, 1024
NH, HD = 16, 64
DFF = 4 * C
N_CORES = 8
EPS = 1e-5
LH = NH // 2          # 8 local heads per core
NKT = C // P          # 8 C partition-tiles
NTCH = T // 512       # 4 token chunks over the full sequence
MYT = T // 2          # 1024 tokens owned post-attention
NMCH = MYT // 512     # 2 token chunks over owned tokens
SCALE = HD ** -0.5    # 1/8


def _ln_stats(nc, pool_ps, pool_sb, ones_bf, src_tiles_bf16_fn, nkt, width, tag):
    """All-ones-matmul LN statistics for one 512-token chunk, broadcast form.

    src_tiles_bf16_fn(kt) -> (xb, xsq) bf16 [128, width] tiles for C-tile kt.
    Returns r_b (bf16 [128,width], rstd broadcast) and mu_b (f32 [128,width]).
    """
    ps_s = pool_ps.tile([P, width], F32, name=f"ps_s_{tag}", tag="stat_s", bufs=2)
    ps_q = pool_ps.tile([P, width], F32, name=f"ps_q_{tag}", tag="stat_q", bufs=2)
    for kt in range(nkt):
        xb, xsq = src_tiles_bf16_fn(kt)
        nc.tensor.matmul(ps_s[:, :], ones_bf[:, 0:P], xb,
                         start=(kt == 0), stop=(kt == nkt - 1))
        nc.tensor.matmul(ps_q[:, :], ones_bf[:, 0:P], xsq,
                         start=(kt == 0), stop=(kt == nkt - 1))
    mu_b = pool_sb.tile([P, width], F32, name=f"mu_b_{tag}", tag="mu_b", bufs=2)
    nc.vector.tensor_scalar(mu_b[:, :], ps_s[:, :], 1.0 / C, None, ALU.mult)
    ve = pool_sb.tile([P, width], F32, name=f"ve_{tag}", tag="ve", bufs=2)
    # E[x^2] + eps
    nc.vector.tensor_scalar(ve[:, :], ps_q[:, :], 1.0 / C, EPS, ALU.mult, ALU.add)
    m2 = pool_sb.tile([P, width], F32, name=f"m2_{tag}", tag="m2", bufs=2)
    nc.vector.tensor_tensor(m2[:, :], mu_b[:, :], mu_b[:, :], ALU.mult)
    nc.vector.tensor_tensor(ve[:, :], ve[:, :], m2[:, :], ALU.subtract)
    # rstd = exp(-0.5 * ln(var + eps))  (ACT Rsqrt is banned for accuracy)
    lnv = pool_sb.tile([P, width], F32, name=f"lnv_{tag}", tag="lnv", bufs=2)
    nc.scalar.activation(lnv[:, :], ve[:, :], AF.Ln)
    r_b = pool_sb.tile([P, width], BF16, name=f"r_b_{tag}", tag="r_b", bufs=2)
    nc.scalar.activation(r_b[:, :], lnv[:, :], AF.Exp, scale=-0.5)
    return r_b, mu_b


def build(tc):
    nc = tc.nc
    ctx = ExitStack()

    # ---------------- external IO (per-core, program-identical) -------------
    xT = nc.dram_tensor("xT", [C, T], F32, kind="ExternalInput").ap()
    xmT = nc.dram_tensor("xmT", [C, MYT], F32, kind="ExternalInput").ap()
    wqkvT = nc.dram_tensor("wqkvT", [C, 3 * 512], BF16, kind="ExternalInput").ap()
    wprojT = nc.dram_tensor("wprojT", [C, C], BF16, kind="ExternalInput").ap()
    wff1T = nc.dram_tensor("wff1T", [C, DFF], BF16, kind="ExternalInput").ap()
    wff2T = nc.dram_tensor("wff2T", [DFF, C], BF16, kind="ExternalInput").ap()
    ln1w = nc.dram_tensor("ln1w", [P, NKT], F32, kind="ExternalInput").ap()
    ln1b = nc.dram_tensor("ln1b", [P, NKT], F32, kind="ExternalInput").ap()
    ln2w = nc.dram_tensor("ln2w", [P, NKT], F32, kind="ExternalInput").ap()
    ln2b = nc.dram_tensor("ln2b", [P, NKT], F32, kind="ExternalInput").ap()
    fb1 = nc.dram_tensor("fb1", [P, DFF // P], F32, kind="ExternalInput").ap()
    fb2 = nc.dram_tensor("fb2", [P, NKT], F32, kind="ExternalInput").ap()
    # per-core head-group row masks: cols 0:64 = (g==0), cols 64:128 = (g==1)
    mrow = nc.dram_tensor("mrow", [1, P], BF16, kind="ExternalInput").ap()
    outT = nc.dram_tensor("outT", [C, MYT], F32, kind="ExternalOutput").ap()

    # ---------------- long-lived SBUF ----------------------------------------
    const = ctx.enter_context(tc.tile_pool(name="const", bufs=1, side="left"))
    ones_bf = const.tile([P, P], BF16, name="ones_bf", tag="ones_bf")
    nc.vector.memset(ones_bf[:, :], 1.0)
    c_ln1w = const.tile([P, NKT], F32, name="c_ln1w", tag="c_ln1w")
    nc.sync.dma_start(out=c_ln1w[:, :], in_=ln1w)
    c_ln1b = const.tile([P, NKT], F32, name="c_ln1b", tag="c_ln1b")
    nc.sync.dma_start(out=c_ln1b[:, :], in_=ln1b)
    c_ln2w = const.tile([P, NKT], F32, name="c_ln2w", tag="c_ln2w")
    nc.sync.dma_start(out=c_ln2w[:, :], in_=ln2w)
    c_ln2b = const.tile([P, NKT], F32, name="c_ln2b", tag="c_ln2b")
    nc.sync.dma_start(out=c_ln2b[:, :], in_=ln2b)
    c_fb1 = const.tile([P, DFF // P], F32, name="c_fb1", tag="c_fb1")
    nc.sync.dma_start(out=c_fb1[:, :], in_=fb1)
    c_fb2 = const.tile([P, NKT], F32, name="c_fb2", tag="c_fb2")
    nc.sync.dma_start(out=c_fb2[:, :], in_=fb2)
    c_mrow = const.tile([1, P], BF16, name="c_mrow", tag="c_mrow")
    nc.sync.dma_start(out=c_mrow[:, :], in_=mrow)

    s_xm = ExitStack()    # xm lives until end of proj
    persist = s_xm.enter_context(
        tc.tile_pool(name="persist", bufs=1, side="right"))
    xm = [persist.tile([P, MYT], F32, name=f"xm{i}", tag=f"xm{i}")
          for i in range(NKT)]
    for i in range(NKT):
        nc.sync.dma_start(out=xm[i][:, :], in_=xmT[i * P:(i + 1) * P, :])

    # ======================= Phase 1: LN1 =====================================
    s_h = ExitStack()     # h lives until end of QKV
    h_pool = s_h.enter_context(tc.tile_pool(name="h_pool", bufs=1, side="left"))
    h = [h_pool.tile([P, T], BF16, name=f"h{i}", tag=f"h{i}") for i in range(NKT)]

    with tc.tile_pool(name="ln1_x", bufs=1, side="left") as xpool, \
         tc.tile_pool(name="ln1_t", bufs=3, side="left") as tpool, \
         tc.tile_pool(name="ln1_s", bufs=2, side="left") as spool, \
         tc.tile_pool(name="ln1_ps", bufs=2, space="PSUM") as pspool:
        x = [xpool.tile([P, T], F32, name=f"x{i}", tag=f"x{i}")
             for i in range(NKT)]
        for i in range(NKT):
            nc.sync.dma_start(out=x[i][:, :], in_=xT[i * P:(i + 1) * P, :])

        for ch in range(NTCH):
            sl = slice(ch * 512, (ch + 1) * 512)

            def _src(kt, sl=sl, ch=ch):
                xb = tpool.tile([P, 512], BF16, name=f"xb_{ch}_{kt}",
                                tag="xb", bufs=3)
                nc.vector.tensor_scalar(xb[:, :], x[kt][:, sl], 1.0, None, ALU.mult)
                xsq = tpool.tile([P, 512], BF16, name=f"xsq_{ch}_{kt}",
                                 tag="xsq", bufs=3)
                nc.vector.tensor_tensor(xsq[:, :], xb[:, :], xb[:, :], ALU.mult)
                return xb, xsq

            r_b, mu_b = _ln_stats(nc, pspool, spool, ones_bf, _src, NKT,
                                  512, f"ln1_{ch}")
            for kt in range(NKT):
                t0 = tpool.tile([P, 512], BF16, name=f"t0_{ch}_{kt}",
                                tag="t0", bufs=3)
                nc.vector.tensor_tensor(t0[:, :], x[kt][:, sl], mu_b[:, :],
                                        ALU.subtract)
                t1 = tpool.tile([P, 512], BF16, name=f"t1_{ch}_{kt}",
                                tag="t1", bufs=3)
                nc.vector.tensor_tensor(t1[:, :], t0[:, :], r_b[:, :], ALU.mult)
                nc.vector.tensor_scalar(h[kt][:, sl], t1[:, :],
                                        c_ln1w[:, kt:kt + 1],
                                        c_ln1b[:, kt:kt + 1],
                                        ALU.mult, ALU.add)

    # =================== Phase 2: QKV projections =============================
    s_qkv = ExitStack()   # qT/kT/v live until end of attention
    qkv_pool = s_qkv.enter_context(tc.tile_pool(name="qkv_out", bufs=1, side="right"))
    qT = [qkv_pool.tile([P, T], BF16, name=f"qT{m}", tag=f"qT{m}")
          for m in range(4)]
    kT = [qkv_pool.tile([P, T], BF16, name=f"kT{m}", tag=f"kT{m}")
          for m in range(4)]
    v = [qkv_pool.tile([P, LH * 65], BF16, name=f"v{mt}", tag=f"v{mt}")
         for mt in range(T // P)]
    for mt in range(T // P):
        nc.vector.memset(v[mt][:, 64::65], 1.0)

    with tc.tile_pool(name="wqkv", bufs=1, side="right") as wpool, \
         tc.tile_pool(name="qkv_ps", bufs=2, space="PSUM") as pspool:
        wqkv = [wpool.tile([P, 3 * 512], BF16, name=f"wqkv{i}", tag=f"wqkv{i}")
                for i in range(NKT)]
        for i in range(NKT):
            nc.sync.dma_start(out=wqkv[i][:, :], in_=wqkvT[i * P:(i + 1) * P, :])

        # q^T, k^T : feature-on-partition via W-stationary matmuls
        for m in range(8):
            dst = qT[m] if m < 4 else kT[m - 4]
            for ch in range(NTCH):
                sl = slice(ch * 512, (ch + 1) * 512)
                ps = pspool.tile([P, 512], F32, name=f"qk_ps_{m}_{ch}",
                                 tag="qk_ps")
                for kt in range(NKT):
                    nc.tensor.matmul(ps[:, :],
                                     wqkv[kt][:, m * P:(m + 1) * P],
                                     h[kt][:, sl],
                                     start=(kt == 0), stop=(kt == NKT - 1))
                nc.scalar.copy(dst[:, sl], ps[:, :])
        # v : token-on-partition via h-stationary matmuls, strided into a
        # [.., 8*65] layout whose column 64 of each head group is ones.
        for mt in range(T // P):
            ps = pspool.tile([P, 512], F32, name=f"v_ps_{mt}", tag="v_ps")
            for kt in range(NKT):
                nc.tensor.matmul(ps[:, :],
                                 h[kt][:, mt * P:(mt + 1) * P],
                                 wqkv[kt][:, 1024:1536],
                                 start=(kt == 0), stop=(kt == NKT - 1))
            dst = v[mt].rearrange("p (h w) -> p h w", w=65)[:, :, 0:64]
            src = ps.rearrange("p (h w) -> p h w", w=64)
            nc.vector.tensor_scalar(dst, src, 1.0, None, ALU.mult)
    s_h.close()

    # ======================= Phase 3: attention ===============================
    # s^T[k,q] blocks with 2 heads packed per 128-row PE pass (K=64 each).
    s_y = ExitStack()
    att_pool = s_y.enter_context(tc.tile_pool(name="att_y", bufs=1, side="left"))
    # per local head: [65, T] bf16; y^T rows 0:64, softmax denominator row 64
    yh = [att_pool.tile([65, T], BF16, name=f"yh{hh}", tag=f"yh{hh}")
          for hh in range(LH)]
    l8 = att_pool.tile([LH, T], BF16, name="l8", tag="l8")

    with tc.tile_pool(name="att_s_ps", bufs=2, space="PSUM") as sps, \
         tc.tile_pool(name="att_y_ps", bufs=2, space="PSUM") as yps, \
         tc.tile_pool(name="att_p", bufs=4, side="left") as ppool:
        for pr in range(4):
            hA, hB = 2 * pr, 2 * pr + 1
            for qc in range(NTCH):
                qsl = slice(qc * 512, (qc + 1) * 512)
                nkt = 4 * qc + 4
                ya = yps.tile([65, 512], F32, name=f"ya_{pr}_{qc}", tag="ya")
                yb = yps.tile([65, 512], F32, name=f"yb_{pr}_{qc}", tag="yb")
                for kt in range(nkt):
                    ksl = slice(kt * P, (kt + 1) * P)
                    diag = (kt >= 4 * qc)
                    for hh, pb, yt in ((hA, 0, ya), (hB, 64, yb)):
                        ss = sps.tile([P, 512], F32,
                                      name=f"ss_{pr}_{qc}_{kt}_{hh}", tag="ss")
                        nc.tensor.matmul(ss[:, :],
                                         kT[pr][pb:pb + 64, ksl],
                                         qT[pr][pb:pb + 64, qsl],
                                         start=True, stop=True)
                        pa = ppool.tile([P, 512], BF16,
                                        name=f"pa_{pr}_{qc}_{kt}_{hh}", tag="pa")
                        nc.scalar.activation(pa[:, :], ss[:, :], AF.Exp,
                                             scale=SCALE)
                        if diag:
                            # keep where q_global - k_global >= 0
                            nc.gpsimd.affine_select(
                                pa[:, :], pa[:, :], pattern=[[1, 512]],
                                compare_op=ALU.is_ge, fill=0.0,
                                base=qc * 512 - kt * P, channel_multiplier=-1)
                        nc.tensor.matmul(yt[:, :],
                                         v[kt][:, hh * 65:(hh + 1) * 65],
                                         pa[:, :],
                                         start=(kt == 0), stop=(kt == nkt - 1))
                nc.scalar.copy(yh[hA][:, qsl], ya[:, :])
                nc.scalar.copy(yh[hB][:, qsl], yb[:, :])

    # gather l rows (partition 64 of each yh) into [8, T];  1/l = exp(-ln l)
    recl8 = att_pool.tile([LH, T], BF16, name="recl8", tag="recl8")
    for hh in range(LH):
        nc.sync.dma_start(out=l8[hh:hh + 1, :], in_=yh[hh][64:65, :])
    nc.scalar.activation(recl8[:, :], l8[:, :], AF.Ln)
    nc.scalar.activation(recl8[:, :], recl8[:, :], AF.Exp, scale=-1.0)
    s_qkv.close()

    # Normalize y^T_h by broadcast(mask/l_h).  The K=1 broadcast matmul's
    # stationary operand is the shipped mask row, so the two masked copies
    # (head-group row region ours / not ours) come out directly: the "B"
    # copy first into stB, then the "A" copy in place of yh.
    stB = [att_pool.tile([64, T], BF16, name=f"stB{hh}", tag=f"stB{hh}")
           for hh in range(LH)]
    with tc.tile_pool(name="nrm_ps", bufs=3, space="PSUM") as bps, \
         tc.tile_pool(name="nrm_st", bufs=3, side="left") as stp:
        for hh in range(LH):
            for ch in range(NTCH):
                sl = slice(ch * 512, (ch + 1) * 512)
                stg = stp.tile([1, 512], BF16, name=f"stg_{hh}_{ch}", tag="stg")
                nc.sync.dma_start(out=stg[0:1, :], in_=recl8[hh:hh + 1, sl])
                bbB = bps.tile([64, 512], F32, name=f"bbB_{hh}_{ch}", tag="bbB")
                nc.tensor.matmul(bbB[:, :], c_mrow[0:1, 64:128], stg[0:1, :],
                                 start=True, stop=True)
                nc.vector.tensor_tensor(stB[hh][:, sl], yh[hh][0:64, sl],
                                        bbB[:, :], ALU.mult)
                bbA = bps.tile([64, 512], F32, name=f"bbA_{hh}_{ch}", tag="bbA")
                nc.tensor.matmul(bbA[:, :], c_mrow[0:1, 0:64], stg[0:1, :],
                                 start=True, stop=True)
                nc.vector.tensor_tensor(yh[hh][0:64, sl], yh[hh][0:64, sl],
                                        bbA[:, :], ALU.mult)

    # ================== Phase 4: pairwise ReduceScatter =======================
    # bounce_in[j*1024 + r*512 + hh*64 + i, q] = contribution to token-half j,
    # head-group r.  Our rows are nonzero only for r == g (mask).  The RS(add)
    # over the pair sums the two cores' buffers and hands shard j to group
    # rank j; since group rank == g == owned token half, each core receives
    # exactly its q-half of the full y^T, rows in global head order.
    s_yf = ExitStack()
    yf_pool = s_yf.enter_context(tc.tile_pool(name="yfull", bufs=1, side="right"))
    yfull = [yf_pool.tile([P, MYT], BF16, name=f"yfull{i}", tag=f"yfull{i}")
             for i in range(NKT)]
    with tc.tile_pool(name="cc_dram", bufs=1, space="DRAM") as dpool:
        bounce_in = dpool.tile([2 * C, MYT], BF16, name="bounce_in", tag="bi")
        bounce_out = dpool.tile([C, MYT], BF16, name="bounce_out", tag="bo")
        for j in range(2):
            for hh in range(LH):
                nc.sync.dma_start(
                    out=bounce_in[j * C + 0 * 512 + hh * 64:
                                  j * C + 0 * 512 + (hh + 1) * 64, :],
                    in_=yh[hh][0:64, j * MYT:(j + 1) * MYT])
                nc.sync.dma_start(
                    out=bounce_in[j * C + 512 + hh * 64:
                                  j * C + 512 + (hh + 1) * 64, :],
                    in_=stB[hh][:, j * MYT:(j + 1) * MYT])
        nc.gpsimd.collective_compute(
            "ReduceScatter", ALU.add,
            replica_groups=[[0, 1], [2, 3], [4, 5], [6, 7]],
            ins=[bounce_in.opt()], outs=[bounce_out.opt()])
        for i in range(NKT):
            nc.sync.dma_start(out=yfull[i][:, :],
                              in_=bounce_out[i * P:(i + 1) * P, :])
    s_y.close()

    # ======================= Phase 5: proj + residual =========================
    s_x2 = ExitStack()
    x2_pool = s_x2.enter_context(tc.tile_pool(name="x2", bufs=1, side="left"))
    x2 = [x2_pool.tile([P, MYT], F32, name=f"x2_{i}", tag=f"x2_{i}")
          for i in range(NKT)]
    with tc.tile_pool(name="wproj", bufs=1, side="right") as wpool, \
         tc.tile_pool(name="proj_ps", bufs=3, space="PSUM") as pspool:
        wproj = [wpool.tile([P, C], BF16, name=f"wproj{i}", tag=f"wproj{i}")
                 for i in range(NKT)]
        for i in range(NKT):
            nc.sync.dma_start(out=wproj[i][:, :], in_=wprojT[i * P:(i + 1) * P, :])
        for m in range(NKT):
            for ch in range(NMCH):
                sl = slice(ch * 512, (ch + 1) * 512)
                ps = pspool.tile([P, 512], F32, name=f"pj_ps_{m}_{ch}", tag="pj")
                for kt in range(NKT):
                    nc.tensor.matmul(ps[:, :], wproj[kt][:, m * P:(m + 1) * P],
                                     yfull[kt][:, sl],
                                     start=(kt == 0), stop=(kt == NKT - 1))
                nc.vector.tensor_tensor(x2[m][:, sl], ps[:, :], xm[m][:, sl],
                                        ALU.add)
    s_yf.close()
    s_xm.close()

    # ======================= Phase 6: LN2 =====================================
    s_h2 = ExitStack()
    h2_pool = s_h2.enter_context(tc.tile_pool(name="h2", bufs=1, side="right"))
    h2 = [h2_pool.tile([P, MYT], BF16, name=f"h2_{i}", tag=f"h2_{i}")
          for i in range(NKT)]
    with tc.tile_pool(name="ln2_t", bufs=3, side="right") as tpool, \
         tc.tile_pool(name="ln2_s", bufs=2, side="right") as spool, \
         tc.tile_pool(name="ln2_ps", bufs=2, space="PSUM") as pspool:
        for ch in range(NMCH):
            sl = slice(ch * 512, (ch + 1) * 512)

            def _src(kt, sl=sl, ch=ch):
                xb = tpool.tile([P, 512], BF16, name=f"x2b_{ch}_{kt}",
                                tag="x2b", bufs=3)
                nc.vector.tensor_scalar(xb[:, :], x2[kt][:, sl], 1.0, None, ALU.mult)
                xsq = tpool.tile([P, 512], BF16, name=f"x2sq_{ch}_{kt}",
                                 tag="x2sq", bufs=3)
                nc.vector.tensor_tensor(xsq[:, :], xb[:, :], xb[:, :], ALU.mult)
                return xb, xsq

            r_b, mu_b = _ln_stats(nc, pspool, spool, ones_bf, _src, NKT,
                                  512, f"ln2_{ch}")
            for kt in range(NKT):
                t0 = tpool.tile([P, 512], BF16, name=f"u0_{ch}_{kt}",
                                tag="u0", bufs=3)
                nc.vector.tensor_tensor(t0[:, :], x2[kt][:, sl], mu_b[:, :],
                                        ALU.subtract)
                t1 = tpool.tile([P, 512], BF16, name=f"u1_{ch}_{kt}",
                                tag="u1", bufs=3)
                nc.vector.tensor_tensor(t1[:, :], t0[:, :], r_b[:, :], ALU.mult)
                nc.vector.tensor_scalar(h2[kt][:, sl], t1[:, :],
                                        c_ln2w[:, kt:kt + 1],
                                        c_ln2b[:, kt:kt + 1],
                                        ALU.mult, ALU.add)

    # ======================= Phase 7: FFN (two dff halves) ====================
    out_pool = s_h2.enter_context(tc.tile_pool(name="out_sb", bufs=1, side="right"))
    acc = [out_pool.tile([P, MYT], F32, name=f"acc{i}", tag=f"acc{i}")
           for i in range(NKT)]
    HKT = DFF // 2 // P  # 16 dff tiles per half
    with tc.tile_pool(name="w1p", bufs=1, side="right") as w1p, \
         tc.tile_pool(name="w2p", bufs=1, side="right") as w2p, \
         tc.tile_pool(name="fp", bufs=1, side="right") as fpool, \
         tc.tile_pool(name="ff1_ps", bufs=2, space="PSUM") as ps1, \
         tc.tile_pool(name="ff2_ps", bufs=2, space="PSUM") as ps2:
        for half in range(2):
            d0 = half * (DFF // 2)
            w1 = [w1p.tile([P, DFF // 2], BF16, name=f"w1_{half}_{i}",
                           tag=f"w1_{i}") for i in range(NKT)]
            for i in range(NKT):
                nc.sync.dma_start(out=w1[i][:, :],
                                  in_=wff1T[i * P:(i + 1) * P, d0:d0 + DFF // 2])
            w2 = [w2p.tile([P, C], BF16, name=f"w2_{half}_{i}", tag=f"w2_{i}")
                  for i in range(HKT)]
            for i in range(HKT):
                nc.sync.dma_start(
                    out=w2[i][:, :],
                    in_=wff2T[d0 + i * P: d0 + (i + 1) * P, :])
            f = [fpool.tile([P, MYT], BF16, name=f"f_{half}_{i}", tag=f"f_{i}")
                 for i in range(HKT)]
            # ff1 + GELU (erf)
            for dt_ in range(HKT):
                ps = ps1.tile([P, MYT], F32, name=f"f1ps_{half}_{dt_}", tag="f1")
                for ch in range(NMCH):
                    sl = slice(ch * 512, (ch + 1) * 512)
                    for kt in range(NKT):
                        nc.tensor.matmul(ps[:, sl],
                                         w1[kt][:, dt_ * P:(dt_ + 1) * P],
                                         h2[kt][:, sl],
                                         start=(kt == 0), stop=(kt == NKT - 1))
                j = d0 // P + dt_
                nc.scalar.activation(f[dt_][:, :], ps[:, :], AF.Gelu,
                                     bias=c_fb1[:, j:j + 1])
            # ff2 partial, accumulated across halves in fp32 SBUF
            for m in range(NKT):
                ps = ps2.tile([P, MYT], F32, name=f"f2ps_{half}_{m}", tag="f2")
                for ch in range(NMCH):
                    sl = slice(ch * 512, (ch + 1) * 512)
                    for kt in range(HKT):
                        nc.tensor.matmul(ps[:, sl], w2[kt][:, m * P:(m + 1) * P],
                                         f[kt][:, sl],
                                         start=(kt == 0), stop=(kt == HKT - 1))
                if half == 0:
                    nc.vector.tensor_scalar(acc[m][:, :], ps[:, :], 1.0, None, ALU.mult)
                else:
                    nc.vector.tensor_tensor(acc[m][:, :], acc[m][:, :],
                                            ps[:, :], ALU.add)
                    nc.vector.tensor_tensor(acc[m][:, :], acc[m][:, :],
                                            x2[m][:, :], ALU.add)
                    nc.scalar.activation(acc[m][:, :], acc[m][:, :],
                                         AF.Identity, bias=c_fb2[:, m:m + 1])
                    nc.sync.dma_start(out=outT[m * P:(m + 1) * P, :],
                                      in_=acc[m][:, :])
    s_x2.close()
    s_h2.close()
    ctx.close()


_CACHED_NC = None


def _get_nc():
    global _CACHED_NC
    if _CACHED_NC is None:
        nc = bass.Bass("TRN2", num_devices=N_CORES)
        with tile.TileContext(nc) as tc:
            build(tc)
        _CACHED_NC = nc
    return _CACHED_NC


def _bf(a):
    return np.ascontiguousarray(a).astype(ml_dtypes.bfloat16)


def make_in_maps(inputs):
    x = np.asarray(inputs["x"], np.float32)
    qkv_w = np.asarray(inputs["qkv_w"], np.float32)
    proj_w = np.asarray(inputs["proj_w"], np.float32)
    ff_w1 = np.asarray(inputs["ff_w1"], np.float32)
    ff_w2 = np.asarray(inputs["ff_w2"], np.float32)

    def vec_tiles(name, n):
        a = np.asarray(inputs[name], np.float32)
        return np.ascontiguousarray(a.reshape(n, P).T)

    wprojT = _bf(proj_w.T)
    wff1T = _bf(ff_w1.T)
    wff2T = _bf(ff_w2.T)
    ln1w = vec_tiles("ln1_w", NKT)
    ln1b = vec_tiles("ln1_b", NKT)
    ln2w = vec_tiles("ln2_w", NKT)
    ln2b = vec_tiles("ln2_b", NKT)
    fb1 = vec_tiles("ff_b1", DFF // P)
    fb2 = vec_tiles("ff_b2", NKT)

    in_maps = []
    for c in range(N_CORES):
        b, g = c // 2, c % 2
        wq = qkv_w[g * 512:(g + 1) * 512, :]
        wk = qkv_w[C + g * 512: C + (g + 1) * 512, :]
        wv = qkv_w[2 * C + g * 512: 2 * C + (g + 1) * 512, :]
        wqkvT = _bf(np.concatenate([wq.T, wk.T, wv.T], axis=1))
        mrow = np.zeros((1, P), np.float32)
        mrow[0, g * 64:(g + 1) * 64] = 1.0
        in_maps.append({
            "xT": np.ascontiguousarray(x[b].T),
            "xmT": np.ascontiguousarray(x[b, g * MYT:(g + 1) * MYT, :].T),
            "wqkvT": wqkvT,
            "wprojT": wprojT,
            "wff1T": wff1T,
            "wff2T": wff2T,
            "ln1w": ln1w, "ln1b": ln1b, "ln2w": ln2w, "ln2b": ln2b,
            "fb1": fb1, "fb2": fb2, "mrow": mrow.astype(ml_dtypes.bfloat16),
        })
    return in_maps


def kernel(**inputs):
    nc = _get_nc()
    in_maps = make_in_maps(inputs)
    res = run_bass_kernel_spmd(
        nc, in_maps, core_ids=list(range(N_CORES)),
        trace=bool(int(os.environ.get("KERNEL_TRACE", "0"))))
    if res.exec_time_ns is not None:
        print(f"HW exec time: {res.exec_time_ns} ns")
    out = np.zeros((B, T, C), np.float32)
    for c in range(N_CORES):
        b, g = c // 2, c % 2
        out[b, g * MYT:(g + 1) * MYT, :] = res.results[c]["outT"].T
    return (out, np.zeros((), np.float32))


if __name__ == "__main__":
    _get_nc()
    print("built ok")


# revision 16
# speedup vs baseline: 1.3474x; 1.3474x over previous
"""Trainium2 Bass kernel for a dense transformer block (B=4, T=2048, C=1024,
16 heads, dff=4096, causal attention, erf-GELU FFN, LN + residuals).

Sharding over 8 NeuronCores: core c handles sequence b = c//2 and head-group
g = c%2 (8 of 16 heads).  Attention is computed head-sharded for all 2048
tokens; a pairwise ReduceScatter re-shards by token so each core runs proj +
LN2 + FFN on its 1024-token half.  All activations live feature-on-partition
("transposed" layout) so no on-chip transposes are needed:

  x^T --LN1--> h^T --W-stationary mm--> q^T,k^T ; --h-stationary mm--> v
  s^T[k,q] = k^T.T @ q^T   (2 heads packed in the 128-row PE array, K=64)
  p^T = exp(s^T/8) masked causally; att@v uses v (with a ones column -> also
  yields the softmax denominator l); y^T normalized via a K=1 broadcast
  matmul of 1/l, then exchanged (ReduceScatter of a zero-masked buffer: the
  row region for the other core's head-group is zeroed by a shipped per-core
  mask, so the program is rank-invariant), then proj/FFN as plain
  W^T-stationary matmuls.  Residual adds stay fp32 end to end.
"""

import os
import numpy as np
import ml_dtypes
from contextlib import ExitStack

import json

import concourse.bass as bass
import concourse.tile as tile
import concourse.bass2jax as bass2jax
from concourse import mybir
from concourse.bass_utils import run_bass_kernel_spmd
from concourse.bass_utils import compile_bir_kernel as _orig_compile_bir_kernel


def _legalize_waits(bir_bytes):
    """walrus's per-instruction sync-wait slots are tight (a DVE op with two
    attached waits fails codegen with 'Too many sync wait commands').  Hoist
    all-but-one wait of every instruction into standalone EventSemaphore
    instructions on the same engine queue, which is semantically identical
    (queues block at the hoisted wait before dispatching the instruction)."""
    j = json.loads(bir_bytes)
    uid = [0]
    for fn in j.get("functions", []):
        for blk in fn.get("blocks", []):
            insts = blk.get("instructions", [])
            out = []
            for ins in insts:
                si = ins.get("sync_info")
                if (si and len(si.get("on_wait", [])) > 1
                        and ins.get("opcode") != "EventSemaphore"):
                    waits = si["on_wait"]
                    for w in waits[:-1]:
                        uid[0] += 1
                        out.append({
                            "debug": ins.get("debug", 0),
                            "engine": ins["engine"],
                            "ins": [], "outs": [],
                            "name": f"hoistw-{uid[0]}",
                            "opcode": "EventSemaphore",
                            "sync_info": {"on_update": [], "on_wait": [w]},
                        })
                    si["on_wait"] = [waits[-1]]
                out.append(ins)
            blk["instructions"] = out
    return json.dumps(j).encode()


def _patched_compile_bir_kernel(bir_bytes, *args, **kwargs):
    return _orig_compile_bir_kernel(_legalize_waits(bir_bytes), *args, **kwargs)


bass2jax.compile_bir_kernel = _patched_compile_bir_kernel

BF16 = mybir.dt.bfloat16
F32 = mybir.dt.float32
AF = mybir.ActivationFunctionType
ALU = mybir.AluOpType

P = 128
B, T, C = 4, 2048, 1024
NH, HD = 16, 64
DFF = 4 * C
N_CORES = 8
EPS = 1e-5
LH = NH // 2          # 8 local heads per core
NKT = C // P          # 8 C partition-tiles
NTCH = T // 512       # 4 token chunks over the full sequence
MYT = T // 2          # 1024 tokens owned post-attention
NMCH = MYT // 512     # 2 token chunks over owned tokens
SCALE = HD ** -0.5    # 1/8


def _ln_stats(nc, pool_ps, pool_sb, ones_bf, src_tiles_bf16_fn, nkt, width, tag):
    """All-ones-matmul LN statistics for one 512-token chunk, broadcast form.

    src_tiles_bf16_fn(kt) -> (xb, xsq) bf16 [128, width] tiles for C-tile kt.
    Returns r_b (bf16 [128,width], rstd broadcast) and mu_b (f32 [128,width]).
    """
    ps_s = pool_ps.tile([P, width], F32, name=f"ps_s_{tag}", tag="stat_s", bufs=2)
    ps_q = pool_ps.tile([P, width], F32, name=f"ps_q_{tag}", tag="stat_q", bufs=2)
    for kt in range(nkt):
        xb, xsq = src_tiles_bf16_fn(kt)
        nc.tensor.matmul(ps_s[:, :], ones_bf[:, 0:P], xb,
                         start=(kt == 0), stop=(kt == nkt - 1))
        nc.tensor.matmul(ps_q[:, :], ones_bf[:, 0:P], xsq,
                         start=(kt == 0), stop=(kt == nkt - 1))
    mu_b = pool_sb.tile([P, width], F32, name=f"mu_b_{tag}", tag="mu_b", bufs=2)
    nc.vector.tensor_scalar(mu_b[:, :], ps_s[:, :], 1.0 / C, None, ALU.mult)
    ve = pool_sb.tile([P, width], F32, name=f"ve_{tag}", tag="ve", bufs=2)
    # E[x^2] + eps
    nc.vector.tensor_scalar(ve[:, :], ps_q[:, :], 1.0 / C, EPS, ALU.mult, ALU.add)
    m2 = pool_sb.tile([P, width], F32, name=f"m2_{tag}", tag="m2", bufs=2)
    nc.vector.tensor_tensor(m2[:, :], mu_b[:, :], mu_b[:, :], ALU.mult)
    nc.vector.tensor_tensor(ve[:, :], ve[:, :], m2[:, :], ALU.subtract)
    # rstd = exp(-0.5 * ln(var + eps))  (ACT Rsqrt is banned for accuracy)
    lnv = pool_sb.tile([P, width], F32, name=f"lnv_{tag}", tag="lnv", bufs=2)
    nc.scalar.activation(lnv[:, :], ve[:, :], AF.Ln)
    r_b = pool_sb.tile([P, width], BF16, name=f"r_b_{tag}", tag="r_b", bufs=2)
    nc.scalar.activation(r_b[:, :], lnv[:, :], AF.Exp, scale=-0.5)
    return r_b, mu_b


def build(tc):
    nc = tc.nc
    ctx = ExitStack()

    # ---------------- external IO (per-core, program-identical) -------------
    xT = nc.dram_tensor("xT", [C, T], BF16, kind="ExternalInput").ap()
    xmT = nc.dram_tensor("xmT", [C, MYT], F32, kind="ExternalInput").ap()
    wqkvT = nc.dram_tensor("wqkvT", [C, 3 * 512], BF16, kind="ExternalInput").ap()
    wprojT = nc.dram_tensor("wprojT", [C, C], BF16, kind="ExternalInput").ap()
    wff1T = nc.dram_tensor("wff1T", [C, DFF], BF16, kind="ExternalInput").ap()
    wff2T = nc.dram_tensor("wff2T", [DFF, C], BF16, kind="ExternalInput").ap()
    ln1w = nc.dram_tensor("ln1w", [P, NKT], F32, kind="ExternalInput").ap()
    ln1b = nc.dram_tensor("ln1b", [P, NKT], F32, kind="ExternalInput").ap()
    ln2w = nc.dram_tensor("ln2w", [P, NKT], F32, kind="ExternalInput").ap()
    ln2b = nc.dram_tensor("ln2b", [P, NKT], F32, kind="ExternalInput").ap()
    fb1 = nc.dram_tensor("fb1", [P, DFF // P], F32, kind="ExternalInput").ap()
    fb2 = nc.dram_tensor("fb2", [P, NKT], F32, kind="ExternalInput").ap()
    # per-core head-group row masks: col 0 = (g==0), col 1 = (g==1)
    mab = nc.dram_tensor("mab", [64, 2], F32, kind="ExternalInput").ap()
    outT = nc.dram_tensor("outT", [C, MYT], F32, kind="ExternalOutput").ap()

    # ---------------- long-lived SBUF ----------------------------------------
    const = ctx.enter_context(tc.tile_pool(name="const", bufs=1, side="left"))
    ones_bf = const.tile([P, P], BF16, name="ones_bf", tag="ones_bf")
    nc.vector.memset(ones_bf[:, :], 1.0)
    c_ln1w = const.tile([P, NKT], F32, name="c_ln1w", tag="c_ln1w")
    nc.sync.dma_start(out=c_ln1w[:, :], in_=ln1w)
    c_ln1b = const.tile([P, NKT], F32, name="c_ln1b", tag="c_ln1b")
    nc.sync.dma_start(out=c_ln1b[:, :], in_=ln1b)
    c_ln2w = const.tile([P, NKT], F32, name="c_ln2w", tag="c_ln2w")
    nc.sync.dma_start(out=c_ln2w[:, :], in_=ln2w)
    c_ln2b = const.tile([P, NKT], F32, name="c_ln2b", tag="c_ln2b")
    nc.sync.dma_start(out=c_ln2b[:, :], in_=ln2b)
    c_fb1 = const.tile([P, DFF // P], F32, name="c_fb1", tag="c_fb1")
    nc.sync.dma_start(out=c_fb1[:, :], in_=fb1)
    c_fb2 = const.tile([P, NKT], F32, name="c_fb2", tag="c_fb2")
    nc.sync.dma_start(out=c_fb2[:, :], in_=fb2)
    c_mab = const.tile([64, 2], F32, name="c_mab", tag="c_mab")
    nc.sync.dma_start(out=c_mab[:, :], in_=mab)

    s_xm = ExitStack()    # xm lives until end of proj
    persist = s_xm.enter_context(
        tc.tile_pool(name="persist", bufs=1, side="right"))
    xm = [persist.tile([P, MYT], F32, name=f"xm{i}", tag=f"xm{i}")
          for i in range(NKT)]

    # ======================= Phase 1: LN1 =====================================
    s_h = ExitStack()     # h lives until end of QKV
    h_pool = s_h.enter_context(tc.tile_pool(name="h_pool", bufs=1, side="left"))
    h = [h_pool.tile([P, T], BF16, name=f"h{i}", tag=f"h{i}") for i in range(NKT)]

    with tc.tile_pool(name="ln1_x", bufs=1, side="left") as xpool, \
         tc.tile_pool(name="ln1_t", bufs=3, side="left") as tpool, \
         tc.tile_pool(name="ln1_s", bufs=2, side="left") as spool, \
         tc.tile_pool(name="ln1_ps", bufs=2, space="PSUM") as pspool:
        x = [xpool.tile([P, T], BF16, name=f"x{i}", tag=f"x{i}")
             for i in range(NKT)]
        for ch in range(NTCH):
            for i in range(NKT):
                nc.sync.dma_start(
                    out=x[i][:, ch * 512:(ch + 1) * 512],
                    in_=xT[i * P:(i + 1) * P, ch * 512:(ch + 1) * 512])

        for ch in range(NTCH):
            sl = slice(ch * 512, (ch + 1) * 512)

            def _src(kt, sl=sl, ch=ch):
                xsq = tpool.tile([P, 512], BF16, name=f"xsq_{ch}_{kt}",
                                 tag="xsq", bufs=3)
                nc.vector.tensor_tensor(xsq[:, :], x[kt][:, sl], x[kt][:, sl],
                                        ALU.mult)
                return x[kt][:, sl], xsq

            r_b, mu_b = _ln_stats(nc, pspool, spool, ones_bf, _src, NKT,
                                  512, f"ln1_{ch}")
            for kt in range(NKT):
                t0 = tpool.tile([P, 512], BF16, name=f"t0_{ch}_{kt}",
                                tag="t0", bufs=3)
                nc.vector.tensor_tensor(t0[:, :], x[kt][:, sl], mu_b[:, :],
                                        ALU.subtract)
                t1 = tpool.tile([P, 512], BF16, name=f"t1_{ch}_{kt}",
                                tag="t1", bufs=3)
                nc.vector.tensor_tensor(t1[:, :], t0[:, :], r_b[:, :], ALU.mult)
                nc.vector.tensor_scalar(h[kt][:, sl], t1[:, :],
                                        c_ln1w[:, kt:kt + 1],
                                        c_ln1b[:, kt:kt + 1],
                                        ALU.mult, ALU.add)

    s_yf = ExitStack()
    yf_pool = s_yf.enter_context(tc.tile_pool(name="yfull", bufs=1, side="right"))
    yfull = [yf_pool.tile([P, MYT], BF16, name=f"yfull{i}", tag=f"yfull{i}")
             for i in range(NKT)]

    # =================== Phase 2: QKV projections =============================
    s_qkv = ExitStack()   # qT/kT/v live until end of attention
    qkv_pool = s_qkv.enter_context(tc.tile_pool(name="qkv_out", bufs=1, side="right"))
    qT = [qkv_pool.tile([P, T], BF16, name=f"qT{m}", tag=f"qT{m}")
          for m in range(4)]
    kT = [qkv_pool.tile([P, T], BF16, name=f"kT{m}", tag=f"kT{m}")
          for m in range(4)]
    v = [qkv_pool.tile([P, LH * 65], BF16, name=f"v{mt}", tag=f"v{mt}")
         for mt in range(T // P)]
    for mt in range(T // P):
        nc.vector.memset(v[mt][:, 64::65], 1.0)

    with tc.tile_pool(name="wqkv", bufs=1, side="right") as wpool, \
         tc.tile_pool(name="qkv_ps", bufs=2, space="PSUM") as pspool:
        wqkv = [wpool.tile([P, 3 * 512], BF16, name=f"wqkv{i}", tag=f"wqkv{i}")
                for i in range(NKT)]
        for i in range(NKT):
            nc.sync.dma_start(out=wqkv[i][:, :], in_=wqkvT[i * P:(i + 1) * P, :])

        # q^T, k^T : feature-on-partition via W-stationary matmuls
        for m in range(8):
            dst = qT[m] if m < 4 else kT[m - 4]
            for ch in range(NTCH):
                sl = slice(ch * 512, (ch + 1) * 512)
                ps = pspool.tile([P, 512], F32, name=f"qk_ps_{m}_{ch}",
                                 tag="qk_ps")
                for kt in range(NKT):
                    nc.tensor.matmul(ps[:, :],
                                     wqkv[kt][:, m * P:(m + 1) * P],
                                     h[kt][:, sl],
                                     start=(kt == 0), stop=(kt == NKT - 1))
                nc.vector.tensor_scalar(dst[:, sl], ps[:, :], 1.0, None,
                                        ALU.mult)
        # v : token-on-partition via h-stationary matmuls, strided into a
        # [.., 8*65] layout whose column 64 of each head group is ones.
        for mt in range(T // P):
            ps = pspool.tile([P, 512], F32, name=f"v_ps_{mt}", tag="v_ps")
            for kt in range(NKT):
                nc.tensor.matmul(ps[:, :],
                                 h[kt][:, mt * P:(mt + 1) * P],
                                 wqkv[kt][:, 1024:1536],
                                 start=(kt == 0), stop=(kt == NKT - 1))
            dst = v[mt].rearrange("p (h w) -> p h w", w=65)[:, :, 0:64]
            src = ps.rearrange("p (h w) -> p h w", w=64)
            nc.vector.tensor_scalar(dst, src, 1.0, None, ALU.mult)
    s_h.close()

    # ======================= Phase 3: attention ===============================
    # s^T[k,q] blocks, 2 heads packed per 128-row PE pass (K=64 each); both
    # heads of a pair share one wide [128,1024] score psum so one ACT Exp op
    # evicts them.  Two pairs interleave per kt step to keep PE fed while ACT
    # runs the exps.  The qc (query-chunk) loop is outermost so each finished
    # 512-column block can be normalized, masked, and handed to a column-split
    # pairwise ReduceScatter that overlaps the remaining attention work.
    s_y = ExitStack()
    att_pool = s_y.enter_context(tc.tile_pool(name="att_y", bufs=1, side="left"))
    # per local head: [65, T] bf16; y^T rows 0:64, softmax denominator row 64
    yh = [att_pool.tile([65, T], BF16, name=f"yh{hh}", tag=f"yh{hh}")
          for hh in range(LH)]
    l8 = att_pool.tile([LH, T], BF16, name="l8", tag="l8")
    recl8 = att_pool.tile([LH, T], BF16, name="recl8", tag="recl8")
    stB = [att_pool.tile([64, T], BF16, name=f"stB{hh}", tag=f"stB{hh}")
           for hh in range(LH)]

    dpool = s_y.enter_context(tc.tile_pool(name="cc_dram", bufs=1, space="DRAM"))
    recl_dram = dpool.tile([LH, T], BF16, name="recl_dram", tag="rd")
    bounce_in = [dpool.tile([2 * C, 512], BF16, name=f"bounce_in{k}",
                            tag=f"bi{k}") for k in range(2)]
    bounce_out = [dpool.tile([C, 512], BF16, name=f"bounce_out{k}",
                             tag=f"bo{k}") for k in range(2)]

    with tc.tile_pool(name="att_s_ps", bufs=1, space="PSUM") as sps, \
         tc.tile_pool(name="att_y_ps", bufs=1, space="PSUM") as yps, \
         tc.tile_pool(name="att_p", bufs=3, side="left") as ppool:
        for qc in (1, 3, 0, 2):
            qsl = slice(qc * 512, (qc + 1) * 512)
            nkt = 4 * qc + 4
            for prg in range(2):
                prs = (2 * prg, 2 * prg + 1)
                ypr = {}
                for pi, pr in enumerate(prs):
                    ypr[pr] = (
                        yps.tile([65, 512], F32, name=f"ya_{pr}_{qc}",
                                 tag=f"y{pi}a"),
                        yps.tile([65, 512], F32, name=f"yb_{pr}_{qc}",
                                 tag=f"y{pi}b"))
                for kt in range(nkt):
                    ksl = slice(kt * P, (kt + 1) * P)
                    diag = (kt >= 4 * qc)
                    pas = {}
                    for pi, pr in enumerate(prs):
                        ss = sps.tile([P, 1024], F32,
                                      name=f"ss_{pr}_{qc}_{kt}", tag=f"ss{pi}")
                        nc.tensor.matmul(ss[:, 0:512],
                                         kT[pr][0:64, ksl],
                                         qT[pr][0:64, qsl],
                                         start=True, stop=True)
                        nc.tensor.matmul(ss[:, 512:1024],
                                         kT[pr][64:128, ksl],
                                         qT[pr][64:128, qsl],
                                         start=True, stop=True)
                        pa = ppool.tile([P, 1024], BF16,
                                        name=f"pa_{pr}_{qc}_{kt}",
                                        tag=f"pa{pi}")
                        nc.scalar.activation(pa[:, :], ss[:, :], AF.Exp,
                                             scale=SCALE)
                        if diag:
                            # keep where q_global - k_global >= 0
                            for hx in range(2):
                                nc.gpsimd.affine_select(
                                    pa[:, hx * 512:(hx + 1) * 512],
                                    pa[:, hx * 512:(hx + 1) * 512],
                                    pattern=[[1, 512]],
                                    compare_op=ALU.is_ge, fill=0.0,
                                    base=qc * 512 - kt * P,
                                    channel_multiplier=-1)
                        pas[pr] = pa
                    for pi, pr in enumerate(prs):
                        ya, yb = ypr[pr]
                        hA, hB = 2 * pr, 2 * pr + 1
                        nc.tensor.matmul(ya[:, :],
                                         v[kt][:, hA * 65:(hA + 1) * 65],
                                         pas[pr][:, 0:512],
                                         start=(kt == 0), stop=(kt == nkt - 1))
                        nc.tensor.matmul(yb[:, :],
                                         v[kt][:, hB * 65:(hB + 1) * 65],
                                         pas[pr][:, 512:1024],
                                         start=(kt == 0), stop=(kt == nkt - 1))
                for pr in prs:
                    ya, yb = ypr[pr]
                    nc.vector.tensor_scalar(yh[2 * pr][:, qsl], ya[:, :],
                                            1.0, None, ALU.mult)
                    nc.vector.tensor_scalar(yh[2 * pr + 1][:, qsl], yb[:, :],
                                            1.0, None, ALU.mult)

            # ---- qc column block finished on all heads: normalize + stage ----
            for hh in range(LH):
                nc.sync.dma_start(out=l8[hh:hh + 1, qsl],
                                  in_=yh[hh][64:65, qsl])
            nc.scalar.activation(recl8[:, qsl], l8[:, qsl], AF.Ln)
            nc.scalar.activation(recl8[:, qsl], recl8[:, qsl], AF.Exp,
                                 scale=-1.0)
            nc.sync.dma_start(out=recl_dram[:, qsl], in_=recl8[:, qsl])
            j, k = qc // 2, qc % 2
            for hh in range(LH):
                # partition-broadcast 1/l_h over the 64 head rows via DMA
                bc = ppool.tile([64, 512], BF16, name=f"bc_{hh}_{qc}",
                                tag="bc", bufs=3)
                bsrc = bass.AP(tensor=recl_dram.tensor,
                               offset=recl_dram.offset + hh * T + qc * 512,
                               ap=[[0, 64], [1, 512]])
                nc.sync.dma_start(out=bc[:, :], in_=bsrc)
                nc.vector.tensor_tensor(yh[hh][0:64, qsl], yh[hh][0:64, qsl],
                                        bc[:, :], ALU.mult)
                nc.vector.tensor_scalar(stB[hh][:, qsl], yh[hh][0:64, qsl],
                                        c_mab[:, 1:2], None, ALU.mult)
                nc.vector.tensor_scalar(yh[hh][0:64, qsl], yh[hh][0:64, qsl],
                                        c_mab[:, 0:1], None, ALU.mult)
                nc.sync.dma_start(
                    out=bounce_in[k][j * C + hh * 64: j * C + (hh + 1) * 64, :],
                    in_=yh[hh][0:64, qsl])
                nc.sync.dma_start(
                    out=bounce_in[k][j * C + 512 + hh * 64:
                                     j * C + 512 + (hh + 1) * 64, :],
                    in_=stB[hh][:, qsl])
            if qc >= 2:
                # both halves' qc%2 columns staged -> exchange this column set
                nc.gpsimd.collective_compute(
                    "ReduceScatter", ALU.add,
                    replica_groups=[[0, 1], [2, 3], [4, 5], [6, 7]],
                    ins=[bounce_in[k].opt()], outs=[bounce_out[k].opt()])
                for i in range(NKT):
                    nc.sync.dma_start(
                        out=yfull[i][:, k * 512:(k + 1) * 512],
                        in_=bounce_out[k][i * P:(i + 1) * P, :])
    s_qkv.close()
    s_y.close()

    # ======================= Phase 5: proj + residual =========================
    s_x2 = ExitStack()
    x2_pool = s_x2.enter_context(tc.tile_pool(name="x2", bufs=1, side="left"))
    x2 = [x2_pool.tile([P, MYT], F32, name=f"x2_{i}", tag=f"x2_{i}")
          for i in range(NKT)]
    with tc.tile_pool(name="wproj", bufs=1, side="right") as wpool, \
         tc.tile_pool(name="proj_ps", bufs=3, space="PSUM") as pspool:
        wproj = [wpool.tile([P, C], BF16, name=f"wproj{i}", tag=f"wproj{i}")
                 for i in range(NKT)]
        for i in range(NKT):
            nc.sync.dma_start(out=xm[i][:, :], in_=xmT[i * P:(i + 1) * P, :])
            nc.sync.dma_start(out=wproj[i][:, :], in_=wprojT[i * P:(i + 1) * P, :])
        for ch in (1, 0):
            for m in range(NKT):
                sl = slice(ch * 512, (ch + 1) * 512)
                ps = pspool.tile([P, 512], F32, name=f"pj_ps_{m}_{ch}", tag="pj")
                for kt in range(NKT):
                    nc.tensor.matmul(ps[:, :], wproj[kt][:, m * P:(m + 1) * P],
                                     yfull[kt][:, sl],
                                     start=(kt == 0), stop=(kt == NKT - 1))
                nc.vector.tensor_tensor(x2[m][:, sl], ps[:, :], xm[m][:, sl],
                                        ALU.add)
    s_yf.close()
    s_xm.close()

    # ======================= Phase 6: LN2 =====================================
    s_h2 = ExitStack()
    h2_pool = s_h2.enter_context(tc.tile_pool(name="h2", bufs=1, side="right"))
    h2 = [h2_pool.tile([P, MYT], BF16, name=f"h2_{i}", tag=f"h2_{i}")
          for i in range(NKT)]
    with tc.tile_pool(name="ln2_t", bufs=3, side="right") as tpool, \
         tc.tile_pool(name="ln2_s", bufs=2, side="right") as spool, \
         tc.tile_pool(name="ln2_ps", bufs=2, space="PSUM") as pspool:
        for ch in range(NMCH):
            sl = slice(ch * 512, (ch + 1) * 512)

            def _src(kt, sl=sl, ch=ch):
                xb = tpool.tile([P, 512], BF16, name=f"x2b_{ch}_{kt}",
                                tag="x2b", bufs=3)
                nc.vector.tensor_scalar(xb[:, :], x2[kt][:, sl], 1.0, None, ALU.mult)
                xsq = tpool.tile([P, 512], BF16, name=f"x2sq_{ch}_{kt}",
                                 tag="x2sq", bufs=3)
                nc.vector.tensor_tensor(xsq[:, :], xb[:, :], xb[:, :], ALU.mult)
                return xb, xsq

            r_b, mu_b = _ln_stats(nc, pspool, spool, ones_bf, _src, NKT,
                                  512, f"ln2_{ch}")
            for kt in range(NKT):
                t0 = tpool.tile([P, 512], BF16, name=f"u0_{ch}_{kt}",
                                tag="u0", bufs=3)
                nc.vector.tensor_tensor(t0[:, :], x2[kt][:, sl], mu_b[:, :],
                                        ALU.subtract)
                t1 = tpool.tile([P, 512], BF16, name=f"u1_{ch}_{kt}",
                                tag="u1", bufs=3)
                nc.vector.tensor_tensor(t1[:, :], t0[:, :], r_b[:, :], ALU.mult)
                nc.vector.tensor_scalar(h2[kt][:, sl], t1[:, :],
                                        c_ln2w[:, kt:kt + 1],
                                        c_ln2b[:, kt:kt + 1],
                                        ALU.mult, ALU.add)

    # ======================= Phase 7: FFN (two dff halves) ====================
    out_pool = s_h2.enter_context(tc.tile_pool(name="out_sb", bufs=1, side="right"))
    acc = [out_pool.tile([P, MYT], F32, name=f"acc{i}", tag=f"acc{i}")
           for i in range(NKT)]
    HKT = DFF // 2 // P  # 16 dff tiles per half
    with tc.tile_pool(name="w1p", bufs=1, side="right") as w1p, \
         tc.tile_pool(name="w2p", bufs=1, side="right") as w2p, \
         tc.tile_pool(name="fp", bufs=1, side="right") as fpool, \
         tc.tile_pool(name="ff1_ps", bufs=2, space="PSUM") as ps1, \
         tc.tile_pool(name="ff2_ps", bufs=2, space="PSUM") as ps2:
        for half in range(2):
            d0 = half * (DFF // 2)
            w1 = [w1p.tile([P, DFF // 2], BF16, name=f"w1_{half}_{i}",
                           tag=f"w1_{i}") for i in range(NKT)]
            for i in range(NKT):
                nc.sync.dma_start(out=w1[i][:, :],
                                  in_=wff1T[i * P:(i + 1) * P, d0:d0 + DFF // 2])
            w2 = [w2p.tile([P, C], BF16, name=f"w2_{half}_{i}", tag=f"w2_{i}")
                  for i in range(HKT)]
            for i in range(HKT):
                nc.sync.dma_start(
                    out=w2[i][:, :],
                    in_=wff2T[d0 + i * P: d0 + (i + 1) * P, :])
            f = [fpool.tile([P, MYT], BF16, name=f"f_{half}_{i}", tag=f"f_{i}")
                 for i in range(HKT)]
            # ff1 + GELU (erf)
            for dt_ in range(HKT):
                ps = ps1.tile([P, MYT], F32, name=f"f1ps_{half}_{dt_}", tag="f1")
                for ch in range(NMCH):
                    sl = slice(ch * 512, (ch + 1) * 512)
                    for kt in range(NKT):
                        nc.tensor.matmul(ps[:, sl],
                                         w1[kt][:, dt_ * P:(dt_ + 1) * P],
                                         h2[kt][:, sl],
                                         start=(kt == 0), stop=(kt == NKT - 1))
                j = d0 // P + dt_
                nc.scalar.activation(f[dt_][:, :], ps[:, :], AF.Gelu,
                                     bias=c_fb1[:, j:j + 1])
            # ff2 partial, accumulated across halves in fp32 SBUF
            for m in range(NKT):
                ps = ps2.tile([P, MYT], F32, name=f"f2ps_{half}_{m}", tag="f2")
                for ch in range(NMCH):
                    sl = slice(ch * 512, (ch + 1) * 512)
                    for kt in range(HKT):
                        nc.tensor.matmul(ps[:, sl], w2[kt][:, m * P:(m + 1) * P],
                                         f[kt][:, sl],
                                         start=(kt == 0), stop=(kt == HKT - 1))
                if half == 0:
                    nc.vector.tensor_scalar(acc[m][:, :], ps[:, :], 1.0, None, ALU.mult)
                else:
                    nc.vector.tensor_tensor(acc[m][:, :], acc[m][:, :],
                                            ps[:, :], ALU.add)
                    nc.vector.tensor_tensor(acc[m][:, :], acc[m][:, :],
                                            x2[m][:, :], ALU.add)
                    nc.scalar.activation(acc[m][:, :], acc[m][:, :],
                                         AF.Identity, bias=c_fb2[:, m:m + 1])
                    nc.sync.dma_start(out=outT[m * P:(m + 1) * P, :],
                                      in_=acc[m][:, :])
    s_x2.close()
    s_h2.close()
    ctx.close()


_CACHED_NC = None


def _get_nc():
    global _CACHED_NC
    if _CACHED_NC is None:
        nc = bass.Bass("TRN2", num_devices=N_CORES)
        with tile.TileContext(nc) as tc:
            build(tc)
        _CACHED_NC = nc
    return _CACHED_NC


def _bf(a):
    return np.ascontiguousarray(a).astype(ml_dtypes.bfloat16)


def make_in_maps(inputs):
    x = np.asarray(inputs["x"], np.float32)
    qkv_w = np.asarray(inputs["qkv_w"], np.float32)
    proj_w = np.asarray(inputs["proj_w"], np.float32)
    ff_w1 = np.asarray(inputs["ff_w1"], np.float32)
    ff_w2 = np.asarray(inputs["ff_w2"], np.float32)

    def vec_tiles(name, n):
        a = np.asarray(inputs[name], np.float32)
        return np.ascontiguousarray(a.reshape(n, P).T)

    wprojT = _bf(proj_w.T)
    wff1T = _bf(ff_w1.T)
    wff2T = _bf(ff_w2.T)
    ln1w = vec_tiles("ln1_w", NKT)
    ln1b = vec_tiles("ln1_b", NKT)
    ln2w = vec_tiles("ln2_w", NKT)
    ln2b = vec_tiles("ln2_b", NKT)
    fb1 = vec_tiles("ff_b1", DFF // P)
    fb2 = vec_tiles("ff_b2", NKT)

    in_maps = []
    for c in range(N_CORES):
        b, g = c // 2, c % 2
        wq = qkv_w[g * 512:(g + 1) * 512, :]
        wk = qkv_w[C + g * 512: C + (g + 1) * 512, :]
        wv = qkv_w[2 * C + g * 512: 2 * C + (g + 1) * 512, :]
        wqkvT = _bf(np.concatenate([wq.T, wk.T, wv.T], axis=1))
        mabv = np.zeros((64, 2), np.float32)
        mabv[:, g] = 1.0
        in_maps.append({
            "xT": _bf(x[b].T),
            "xmT": np.ascontiguousarray(x[b, g * MYT:(g + 1) * MYT, :].T),
            "wqkvT": wqkvT,
            "wprojT": wprojT,
            "wff1T": wff1T,
            "wff2T": wff2T,
            "ln1w": ln1w, "ln1b": ln1b, "ln2w": ln2w, "ln2b": ln2b,
            "fb1": fb1, "fb2": fb2, "mab": mabv,
        })
    return in_maps


LAST_RESULT = None


def kernel(**inputs):
    global LAST_RESULT
    nc = _get_nc()
    in_maps = make_in_maps(inputs)
    res = run_bass_kernel_spmd(
        nc, in_maps, core_ids=list(range(N_CORES)),
        trace=bool(int(os.environ.get("KERNEL_TRACE", "0"))))
    LAST_RESULT = res
    if res.exec_time_ns is not None:
        print(f"HW exec time: {res.exec_time_ns} ns")
    out = np.zeros((B, T, C), np.float32)
    for c in range(N_CORES):
        b, g = c // 2, c % 2
        out[b, g * MYT:(g + 1) * MYT, :] = res.results[c]["outT"].T
    return (out, np.zeros((), np.float32))


if __name__ == "__main__":
    _get_nc()
    print("built ok")


# revision 19
# speedup vs baseline: 1.3984x; 1.0378x over previous
"""Trainium2 Bass kernel for a dense transformer block (B=4, T=2048, C=1024,
16 heads, dff=4096, causal attention, erf-GELU FFN, LN + residuals).

Sharding over 8 NeuronCores: core c handles sequence b = c//2 and head-group
g = c%2 (8 of 16 heads).  Attention is computed head-sharded for all 2048
tokens; a pairwise ReduceScatter re-shards by token so each core runs proj +
LN2 + FFN on its 1024-token half.  All activations live feature-on-partition
("transposed" layout) so no on-chip transposes are needed:

  x^T --LN1--> h^T --W-stationary mm--> q^T,k^T ; --h-stationary mm--> v
  s^T[k,q] = k^T.T @ q^T   (2 heads packed in the 128-row PE array, K=64)
  p^T = exp(s^T/8) masked causally; att@v uses v (with a ones column -> also
  yields the softmax denominator l); y^T normalized via a K=1 broadcast
  matmul of 1/l, then exchanged (ReduceScatter of a zero-masked buffer: the
  row region for the other core's head-group is zeroed by a shipped per-core
  mask, so the program is rank-invariant), then proj/FFN as plain
  W^T-stationary matmuls.  Residual adds stay fp32 end to end.
"""

import os
import numpy as np
import ml_dtypes
from contextlib import ExitStack

import json

import concourse.bass as bass
import concourse.tile as tile
import concourse.bass2jax as bass2jax
from concourse import mybir
from concourse.bass_utils import run_bass_kernel_spmd
from concourse.bass_utils import compile_bir_kernel as _orig_compile_bir_kernel


def _legalize_waits(bir_bytes):
    """walrus's per-instruction sync-wait slots are tight (a DVE op with two
    attached waits fails codegen with 'Too many sync wait commands').  Hoist
    all-but-one wait of every instruction into standalone EventSemaphore
    instructions on the same engine queue, which is semantically identical
    (queues block at the hoisted wait before dispatching the instruction)."""
    j = json.loads(bir_bytes)
    uid = [0]
    for fn in j.get("functions", []):
        for blk in fn.get("blocks", []):
            insts = blk.get("instructions", [])
            out = []
            for ins in insts:
                si = ins.get("sync_info")
                if (si and len(si.get("on_wait", [])) > 1
                        and ins.get("opcode") != "EventSemaphore"):
                    waits = si["on_wait"]
                    for w in waits[:-1]:
                        uid[0] += 1
                        out.append({
                            "debug": ins.get("debug", 0),
                            "engine": ins["engine"],
                            "ins": [], "outs": [],
                            "name": f"hoistw-{uid[0]}",
                            "opcode": "EventSemaphore",
                            "sync_info": {"on_update": [], "on_wait": [w]},
                        })
                    si["on_wait"] = [waits[-1]]
                out.append(ins)
            blk["instructions"] = out
    return json.dumps(j).encode()


def _patched_compile_bir_kernel(bir_bytes, *args, **kwargs):
    return _orig_compile_bir_kernel(_legalize_waits(bir_bytes), *args, **kwargs)


bass2jax.compile_bir_kernel = _patched_compile_bir_kernel

BF16 = mybir.dt.bfloat16
F32 = mybir.dt.float32
AF = mybir.ActivationFunctionType
ALU = mybir.AluOpType

P = 128
B, T, C = 4, 2048, 1024
NH, HD = 16, 64
DFF = 4 * C
N_CORES = 8
EPS = 1e-5
LH = NH // 2          # 8 local heads per core
NKT = C // P          # 8 C partition-tiles
NTCH = T // 512       # 4 token chunks over the full sequence
MYT = T // 2          # 1024 tokens owned post-attention
NMCH = MYT // 512     # 2 token chunks over owned tokens
SCALE = HD ** -0.5    # 1/8


def _ln_stats(nc, pool_ps, pool_sb, ones_bf, src_tiles_bf16_fn, nkt, width, tag):
    """All-ones-matmul LN statistics for one 512-token chunk, broadcast form.

    src_tiles_bf16_fn(kt) -> (xb, xsq) bf16 [128, width] tiles for C-tile kt.
    Returns r_b (bf16 [128,width], rstd broadcast) and mu_b (f32 [128,width]).
    """
    ps_s = pool_ps.tile([P, width], F32, name=f"ps_s_{tag}", tag="stat_s", bufs=2)
    ps_q = pool_ps.tile([P, width], F32, name=f"ps_q_{tag}", tag="stat_q", bufs=2)
    for kt in range(nkt):
        xb, xsq = src_tiles_bf16_fn(kt)
        nc.tensor.matmul(ps_s[:, :], ones_bf[:, 0:P], xb,
                         start=(kt == 0), stop=(kt == nkt - 1))
        nc.tensor.matmul(ps_q[:, :], ones_bf[:, 0:P], xsq,
                         start=(kt == 0), stop=(kt == nkt - 1))
    mu_b = pool_sb.tile([P, width], F32, name=f"mu_b_{tag}", tag="mu_b", bufs=2)
    nc.vector.tensor_scalar(mu_b[:, :], ps_s[:, :], 1.0 / C, None, ALU.mult)
    ve = pool_sb.tile([P, width], F32, name=f"ve_{tag}", tag="ve", bufs=2)
    # E[x^2] + eps
    nc.vector.tensor_scalar(ve[:, :], ps_q[:, :], 1.0 / C, EPS, ALU.mult, ALU.add)
    m2 = pool_sb.tile([P, width], F32, name=f"m2_{tag}", tag="m2", bufs=2)
    nc.vector.tensor_tensor(m2[:, :], mu_b[:, :], mu_b[:, :], ALU.mult)
    nc.vector.tensor_tensor(ve[:, :], ve[:, :], m2[:, :], ALU.subtract)
    # rstd = exp(-0.5 * ln(var + eps))  (ACT Rsqrt is banned for accuracy)
    lnv = pool_sb.tile([P, width], F32, name=f"lnv_{tag}", tag="lnv", bufs=2)
    nc.scalar.activation(lnv[:, :], ve[:, :], AF.Ln)
    r_b = pool_sb.tile([P, width], BF16, name=f"r_b_{tag}", tag="r_b", bufs=2)
    nc.scalar.activation(r_b[:, :], lnv[:, :], AF.Exp, scale=-0.5)
    return r_b, mu_b


def build(tc):
    nc = tc.nc
    ctx = ExitStack()

    # ---------------- external IO (per-core, program-identical) -------------
    xT = nc.dram_tensor("xT", [C, T], BF16, kind="ExternalInput").ap()
    xmT = nc.dram_tensor("xmT", [C, MYT], F32, kind="ExternalInput").ap()
    wqkvT = nc.dram_tensor("wqkvT", [C, 3 * 512], BF16, kind="ExternalInput").ap()
    wprojT = nc.dram_tensor("wprojT", [C, C], BF16, kind="ExternalInput").ap()
    wff1T = nc.dram_tensor("wff1T", [C, DFF], BF16, kind="ExternalInput").ap()
    wff2T = nc.dram_tensor("wff2T", [DFF, C], BF16, kind="ExternalInput").ap()
    ln1w = nc.dram_tensor("ln1w", [P, NKT], F32, kind="ExternalInput").ap()
    ln1b = nc.dram_tensor("ln1b", [P, NKT], F32, kind="ExternalInput").ap()
    ln2w = nc.dram_tensor("ln2w", [P, NKT], F32, kind="ExternalInput").ap()
    ln2b = nc.dram_tensor("ln2b", [P, NKT], F32, kind="ExternalInput").ap()
    fb1 = nc.dram_tensor("fb1", [P, DFF // P], F32, kind="ExternalInput").ap()
    fb2 = nc.dram_tensor("fb2", [P, NKT], F32, kind="ExternalInput").ap()
    # per-core head-group row masks: col 0 = (g==0), col 1 = (g==1)
    mab = nc.dram_tensor("mab", [64, 2], F32, kind="ExternalInput").ap()
    outT = nc.dram_tensor("outT", [C, MYT], F32, kind="ExternalOutput").ap()

    # ---------------- long-lived SBUF ----------------------------------------
    const = ctx.enter_context(tc.tile_pool(name="const", bufs=1, side="left"))
    ones_bf = const.tile([P, P], BF16, name="ones_bf", tag="ones_bf")
    nc.vector.memset(ones_bf[:, :], 1.0)
    c_ln1w = const.tile([P, NKT], F32, name="c_ln1w", tag="c_ln1w")
    nc.sync.dma_start(out=c_ln1w[:, :], in_=ln1w)
    c_ln1b = const.tile([P, NKT], F32, name="c_ln1b", tag="c_ln1b")
    nc.sync.dma_start(out=c_ln1b[:, :], in_=ln1b)
    c_ln2w = const.tile([P, NKT], F32, name="c_ln2w", tag="c_ln2w")
    nc.sync.dma_start(out=c_ln2w[:, :], in_=ln2w)
    c_ln2b = const.tile([P, NKT], F32, name="c_ln2b", tag="c_ln2b")
    nc.sync.dma_start(out=c_ln2b[:, :], in_=ln2b)
    c_fb1 = const.tile([P, DFF // P], F32, name="c_fb1", tag="c_fb1")
    nc.sync.dma_start(out=c_fb1[:, :], in_=fb1)
    c_fb2 = const.tile([P, NKT], F32, name="c_fb2", tag="c_fb2")
    nc.sync.dma_start(out=c_fb2[:, :], in_=fb2)
    c_mab = const.tile([64, 2], F32, name="c_mab", tag="c_mab")
    nc.sync.dma_start(out=c_mab[:, :], in_=mab)

    s_xm = ExitStack()    # xm lives until end of proj
    persist = s_xm.enter_context(
        tc.tile_pool(name="persist", bufs=1, side="right"))
    xm = [persist.tile([P, MYT], F32, name=f"xm{i}", tag=f"xm{i}")
          for i in range(NKT)]
    wproj = [persist.tile([P, C], BF16, name=f"wproj{i}", tag=f"wproj{i}")
             for i in range(NKT)]

    # ======================= Phase 1: LN1 =====================================
    s_h = ExitStack()     # h lives until end of QKV
    h_pool = s_h.enter_context(tc.tile_pool(name="h_pool", bufs=1, side="left"))
    h = [h_pool.tile([P, T], BF16, name=f"h{i}", tag=f"h{i}") for i in range(NKT)]

    with tc.tile_pool(name="ln1_x", bufs=1, side="left") as xpool, \
         tc.tile_pool(name="ln1_t", bufs=3, side="left") as tpool, \
         tc.tile_pool(name="ln1_s", bufs=2, side="left") as spool, \
         tc.tile_pool(name="ln1_ps", bufs=2, space="PSUM") as pspool:
        x = [xpool.tile([P, T], BF16, name=f"x{i}", tag=f"x{i}")
             for i in range(NKT)]
        for ch in range(NTCH):
            for i in range(NKT):
                nc.sync.dma_start(
                    out=x[i][:, ch * 512:(ch + 1) * 512],
                    in_=xT[i * P:(i + 1) * P, ch * 512:(ch + 1) * 512])

        for ch in range(NTCH):
            sl = slice(ch * 512, (ch + 1) * 512)

            def _src(kt, sl=sl, ch=ch):
                xsq = tpool.tile([P, 512], BF16, name=f"xsq_{ch}_{kt}",
                                 tag="xsq", bufs=3)
                nc.vector.tensor_tensor(xsq[:, :], x[kt][:, sl], x[kt][:, sl],
                                        ALU.mult)
                return x[kt][:, sl], xsq

            r_b, mu_b = _ln_stats(nc, pspool, spool, ones_bf, _src, NKT,
                                  512, f"ln1_{ch}")
            for kt in range(NKT):
                t0 = tpool.tile([P, 512], BF16, name=f"t0_{ch}_{kt}",
                                tag="t0", bufs=3)
                nc.vector.tensor_tensor(t0[:, :], x[kt][:, sl], mu_b[:, :],
                                        ALU.subtract)
                t1 = tpool.tile([P, 512], BF16, name=f"t1_{ch}_{kt}",
                                tag="t1", bufs=3)
                nc.vector.tensor_tensor(t1[:, :], t0[:, :], r_b[:, :], ALU.mult)
                nc.vector.tensor_scalar(h[kt][:, sl], t1[:, :],
                                        c_ln1w[:, kt:kt + 1],
                                        c_ln1b[:, kt:kt + 1],
                                        ALU.mult, ALU.add)

    s_yf = ExitStack()
    yf_pool = s_yf.enter_context(tc.tile_pool(name="yfull", bufs=1, side="right"))
    yfull = [yf_pool.tile([P, MYT], BF16, name=f"yfull{i}", tag=f"yfull{i}")
             for i in range(NKT)]

    # =================== Phase 2: QKV projections =============================
    s_qkv = ExitStack()   # qT/kT/v live until end of attention
    qkv_pool = s_qkv.enter_context(tc.tile_pool(name="qkv_out", bufs=1, side="right"))
    qT = [qkv_pool.tile([P, T], BF16, name=f"qT{m}", tag=f"qT{m}")
          for m in range(4)]
    kT = [qkv_pool.tile([P, T], BF16, name=f"kT{m}", tag=f"kT{m}")
          for m in range(4)]
    v = [qkv_pool.tile([P, LH * 65], BF16, name=f"v{mt}", tag=f"v{mt}")
         for mt in range(T // P)]
    for mt in range(T // P):
        nc.vector.memset(v[mt][:, 64::65], 1.0)

    with tc.tile_pool(name="wqkv", bufs=1, side="right") as wpool, \
         tc.tile_pool(name="qkv_ps", bufs=2, space="PSUM") as pspool:
        wqkv = [wpool.tile([P, 3 * 512], BF16, name=f"wqkv{i}", tag=f"wqkv{i}")
                for i in range(NKT)]
        for i in range(NKT):
            nc.sync.dma_start(out=wqkv[i][:, :], in_=wqkvT[i * P:(i + 1) * P, :])
        for i in range(NKT):
            nc.sync.dma_start(out=xm[i][:, :], in_=xmT[i * P:(i + 1) * P, :])
            nc.sync.dma_start(out=wproj[i][:, :],
                              in_=wprojT[i * P:(i + 1) * P, :])

        # q^T, k^T : feature-on-partition via W-stationary matmuls
        for m in range(8):
            dst = qT[m] if m < 4 else kT[m - 4]
            for ch in range(NTCH):
                sl = slice(ch * 512, (ch + 1) * 512)
                ps = pspool.tile([P, 512], F32, name=f"qk_ps_{m}_{ch}",
                                 tag="qk_ps")
                for kt in range(NKT):
                    nc.tensor.matmul(ps[:, :],
                                     wqkv[kt][:, m * P:(m + 1) * P],
                                     h[kt][:, sl],
                                     start=(kt == 0), stop=(kt == NKT - 1))
                nc.vector.tensor_scalar(dst[:, sl], ps[:, :], 1.0, None,
                                        ALU.mult)
        # v : token-on-partition via h-stationary matmuls, strided into a
        # [.., 8*65] layout whose column 64 of each head group is ones.
        for mt in range(T // P):
            ps = pspool.tile([P, 512], F32, name=f"v_ps_{mt}", tag="v_ps")
            for kt in range(NKT):
                nc.tensor.matmul(ps[:, :],
                                 h[kt][:, mt * P:(mt + 1) * P],
                                 wqkv[kt][:, 1024:1536],
                                 start=(kt == 0), stop=(kt == NKT - 1))
            dst = v[mt].rearrange("p (h w) -> p h w", w=65)[:, :, 0:64]
            src = ps.rearrange("p (h w) -> p h w", w=64)
            nc.vector.tensor_scalar(dst, src, 1.0, None, ALU.mult)
    s_h.close()

    # ======================= Phase 3: attention ===============================
    # s^T[k,q] blocks, 2 heads packed per 128-row PE pass (K=64 each); both
    # heads of a pair share one wide [128,1024] score psum so one ACT Exp op
    # evicts them.  Two pairs interleave per kt step to keep PE fed while ACT
    # runs the exps.  The qc (query-chunk) loop is outermost so each finished
    # 512-column block can be normalized, masked, and handed to a column-split
    # pairwise ReduceScatter that overlaps the remaining attention work.
    s_y = ExitStack()
    att_pool = s_y.enter_context(tc.tile_pool(name="att_y", bufs=1, side="left"))
    # per local head: [65, T] bf16; y^T rows 0:64, softmax denominator row 64
    yh = [att_pool.tile([65, T], BF16, name=f"yh{hh}", tag=f"yh{hh}")
          for hh in range(LH)]
    l8 = att_pool.tile([LH, T], BF16, name="l8", tag="l8")
    recl8 = att_pool.tile([LH, T], BF16, name="recl8", tag="recl8")

    dpool = s_y.enter_context(tc.tile_pool(name="cc_dram", bufs=1, space="DRAM"))
    recl_dram = dpool.tile([LH, T], BF16, name="recl_dram", tag="rd")
    bounce_in = [dpool.tile([2 * C, 512], BF16, name=f"bounce_in{k}",
                            tag=f"bi{k}") for k in range(2)]
    bounce_out = [dpool.tile([C, 512], BF16, name=f"bounce_out{k}",
                             tag=f"bo{k}") for k in range(2)]

    with tc.tile_pool(name="att_s_ps", bufs=1, space="PSUM") as sps, \
         tc.tile_pool(name="att_y_ps", bufs=1, space="PSUM") as yps, \
         tc.tile_pool(name="att_p", bufs=3, side="left") as ppool:
        for qc in (1, 3, 0, 2):
            qsl = slice(qc * 512, (qc + 1) * 512)
            nkt = 4 * qc + 4
            for prg in range(2):
                prs = (2 * prg, 2 * prg + 1)
                ypr = {}
                for pi, pr in enumerate(prs):
                    ypr[pr] = (
                        yps.tile([65, 512], F32, name=f"ya_{pr}_{qc}",
                                 tag=f"y{pi}a"),
                        yps.tile([65, 512], F32, name=f"yb_{pr}_{qc}",
                                 tag=f"y{pi}b"))
                for kt in range(nkt):
                    ksl = slice(kt * P, (kt + 1) * P)
                    diag = (kt >= 4 * qc)
                    pas = {}
                    for pi, pr in enumerate(prs):
                        ss = sps.tile([P, 1024], F32,
                                      name=f"ss_{pr}_{qc}_{kt}", tag=f"ss{pi}")
                        nc.tensor.matmul(ss[:, 0:512],
                                         kT[pr][0:64, ksl],
                                         qT[pr][0:64, qsl],
                                         start=True, stop=True)
                        nc.tensor.matmul(ss[:, 512:1024],
                                         kT[pr][64:128, ksl],
                                         qT[pr][64:128, qsl],
                                         start=True, stop=True)
                        pa = ppool.tile([P, 1024], BF16,
                                        name=f"pa_{pr}_{qc}_{kt}",
                                        tag=f"pa{pi}")
                        nc.scalar.activation(pa[:, :], ss[:, :], AF.Exp,
                                             scale=SCALE)
                        if diag:
                            # keep where q_global - k_global >= 0
                            for hx in range(2):
                                nc.gpsimd.affine_select(
                                    pa[:, hx * 512:(hx + 1) * 512],
                                    pa[:, hx * 512:(hx + 1) * 512],
                                    pattern=[[1, 512]],
                                    compare_op=ALU.is_ge, fill=0.0,
                                    base=qc * 512 - kt * P,
                                    channel_multiplier=-1)
                        pas[pr] = pa
                    for pi, pr in enumerate(prs):
                        ya, yb = ypr[pr]
                        hA, hB = 2 * pr, 2 * pr + 1
                        nc.tensor.matmul(ya[:, :],
                                         v[kt][:, hA * 65:(hA + 1) * 65],
                                         pas[pr][:, 0:512],
                                         start=(kt == 0), stop=(kt == nkt - 1))
                        nc.tensor.matmul(yb[:, :],
                                         v[kt][:, hB * 65:(hB + 1) * 65],
                                         pas[pr][:, 512:1024],
                                         start=(kt == 0), stop=(kt == nkt - 1))
                for pr in prs:
                    ya, yb = ypr[pr]
                    nc.vector.tensor_scalar(yh[2 * pr][:, qsl], ya[:, :],
                                            1.0, None, ALU.mult)
                    nc.vector.tensor_scalar(yh[2 * pr + 1][:, qsl], yb[:, :],
                                            1.0, None, ALU.mult)

            # ---- qc column block finished on all heads: normalize + stage ----
            for hh in range(LH):
                nc.sync.dma_start(out=l8[hh:hh + 1, qsl],
                                  in_=yh[hh][64:65, qsl])
            nc.scalar.activation(recl8[:, qsl], l8[:, qsl], AF.Ln)
            nc.scalar.activation(recl8[:, qsl], recl8[:, qsl], AF.Exp,
                                 scale=-1.0)
            nc.sync.dma_start(out=recl_dram[:, qsl], in_=recl8[:, qsl])
            j, k = qc // 2, qc % 2
            for hh in range(LH):
                # partition-broadcast 1/l_h over the 64 head rows via DMA
                bc = ppool.tile([64, 512], BF16, name=f"bc_{hh}_{qc}",
                                tag="bc", bufs=3)
                bsrc = bass.AP(tensor=recl_dram.tensor,
                               offset=recl_dram.offset + hh * T + qc * 512,
                               ap=[[0, 64], [1, 512]])
                nc.sync.dma_start(out=bc[:, :], in_=bsrc)
                nc.vector.tensor_tensor(yh[hh][0:64, qsl], yh[hh][0:64, qsl],
                                        bc[:, :], ALU.mult)
                stBt = ppool.tile([64, 512], BF16, name=f"stB_{hh}_{qc}",
                                  tag="stB", bufs=4)
                nc.vector.tensor_scalar(stBt[:, :], yh[hh][0:64, qsl],
                                        c_mab[:, 1:2], None, ALU.mult)
                nc.vector.tensor_scalar(yh[hh][0:64, qsl], yh[hh][0:64, qsl],
                                        c_mab[:, 0:1], None, ALU.mult)
                nc.sync.dma_start(
                    out=bounce_in[k][j * C + hh * 64: j * C + (hh + 1) * 64, :],
                    in_=yh[hh][0:64, qsl])
                nc.sync.dma_start(
                    out=bounce_in[k][j * C + 512 + hh * 64:
                                     j * C + 512 + (hh + 1) * 64, :],
                    in_=stBt[:, :])
            if qc >= 2:
                # both halves' qc%2 columns staged -> exchange this column set
                nc.gpsimd.collective_compute(
                    "ReduceScatter", ALU.add,
                    replica_groups=[[0, 1], [2, 3], [4, 5], [6, 7]],
                    ins=[bounce_in[k].opt()], outs=[bounce_out[k].opt()])
                for i in range(NKT):
                    nc.sync.dma_start(
                        out=yfull[i][:, k * 512:(k + 1) * 512],
                        in_=bounce_out[k][i * P:(i + 1) * P, :])
    s_qkv.close()
    s_y.close()

    # ========== Phases 5-7: proj + LN2 + FFN, column-chunk-major ==============
    # ch=1 (RS_1's columns, exchanged first) flows through proj -> LN2 while
    # RS_0 is still in flight, hiding the exchange tail under PE work.
    s_x2 = ExitStack()
    x2_pool = s_x2.enter_context(tc.tile_pool(name="x2", bufs=1, side="left"))
    x2 = [x2_pool.tile([P, MYT], F32, name=f"x2_{i}", tag=f"x2_{i}")
          for i in range(NKT)]
    s_h2 = ExitStack()
    h2_pool = s_h2.enter_context(tc.tile_pool(name="h2", bufs=1, side="left"))
    h2 = [h2_pool.tile([P, MYT], BF16, name=f"h2_{i}", tag=f"h2_{i}")
          for i in range(NKT)]
    with tc.tile_pool(name="proj_ps", bufs=2, space="PSUM") as pjps, \
         tc.tile_pool(name="ln2_t", bufs=3, side="right") as tpool, \
         tc.tile_pool(name="ln2_s", bufs=2, side="right") as spool, \
         tc.tile_pool(name="ln2_ps", bufs=2, space="PSUM") as lnps:
        for ch in (1, 0):
            sl = slice(ch * 512, (ch + 1) * 512)
            for m in range(NKT):
                ps = pjps.tile([P, 512], F32, name=f"pj_ps_{m}_{ch}", tag="pj")
                for kt in range(NKT):
                    nc.tensor.matmul(ps[:, :], wproj[kt][:, m * P:(m + 1) * P],
                                     yfull[kt][:, sl],
                                     start=(kt == 0), stop=(kt == NKT - 1))
                nc.vector.tensor_tensor(x2[m][:, sl], ps[:, :], xm[m][:, sl],
                                        ALU.add)

            def _src(kt, sl=sl, ch=ch):
                xb = tpool.tile([P, 512], BF16, name=f"x2b_{ch}_{kt}",
                                tag="x2b", bufs=3)
                nc.vector.tensor_scalar(xb[:, :], x2[kt][:, sl], 1.0, None,
                                        ALU.mult)
                xsq = tpool.tile([P, 512], BF16, name=f"x2sq_{ch}_{kt}",
                                 tag="x2sq", bufs=3)
                nc.vector.tensor_tensor(xsq[:, :], xb[:, :], xb[:, :], ALU.mult)
                return xb, xsq

            r_b, mu_b = _ln_stats(nc, lnps, spool, ones_bf, _src, NKT,
                                  512, f"ln2_{ch}")
            for kt in range(NKT):
                t0 = tpool.tile([P, 512], BF16, name=f"u0_{ch}_{kt}",
                                tag="u0", bufs=3)
                nc.vector.tensor_tensor(t0[:, :], x2[kt][:, sl], mu_b[:, :],
                                        ALU.subtract)
                t1 = tpool.tile([P, 512], BF16, name=f"u1_{ch}_{kt}",
                                tag="u1", bufs=3)
                nc.vector.tensor_tensor(t1[:, :], t0[:, :], r_b[:, :], ALU.mult)
                nc.vector.tensor_scalar(h2[kt][:, sl], t1[:, :],
                                        c_ln2w[:, kt:kt + 1],
                                        c_ln2b[:, kt:kt + 1],
                                        ALU.mult, ALU.add)
    s_yf.close()
    s_xm.close()

    # ======================= FFN (two dff halves, ch-major) ===================
    out_pool = s_h2.enter_context(tc.tile_pool(name="out_sb", bufs=1,
                                               side="left"))
    acc = [out_pool.tile([P, MYT], F32, name=f"acc{i}", tag=f"acc{i}")
           for i in range(NKT)]
    HKT = DFF // 2 // P  # 16 dff tiles per half
    with tc.tile_pool(name="w1p", bufs=1, side="right") as w1p, \
         tc.tile_pool(name="w2p", bufs=1, side="right") as w2p, \
         tc.tile_pool(name="fp", bufs=1, side="right") as fpool, \
         tc.tile_pool(name="ff1_ps", bufs=2, space="PSUM") as ps1, \
         tc.tile_pool(name="ff2_ps", bufs=2, space="PSUM") as ps2:
        for half in range(2):
            d0 = half * (DFF // 2)
            w1 = [w1p.tile([P, DFF // 2], BF16, name=f"w1_{half}_{i}",
                           tag=f"w1_{i}") for i in range(NKT)]
            for i in range(NKT):
                nc.sync.dma_start(out=w1[i][:, :],
                                  in_=wff1T[i * P:(i + 1) * P, d0:d0 + DFF // 2])
            w2 = [w2p.tile([P, C], BF16, name=f"w2_{half}_{i}", tag=f"w2_{i}")
                  for i in range(HKT)]
            for i in range(HKT):
                nc.sync.dma_start(
                    out=w2[i][:, :],
                    in_=wff2T[d0 + i * P: d0 + (i + 1) * P, :])
            f = [fpool.tile([P, MYT], BF16, name=f"f_{half}_{i}", tag=f"f_{i}")
                 for i in range(HKT)]
            for ch in (1, 0):
                sl = slice(ch * 512, (ch + 1) * 512)
                # ff1 + GELU (erf)
                for dt_ in range(HKT):
                    ps = ps1.tile([P, 512], F32,
                                  name=f"f1ps_{half}_{dt_}_{ch}", tag="f1")
                    for kt in range(NKT):
                        nc.tensor.matmul(ps[:, :],
                                         w1[kt][:, dt_ * P:(dt_ + 1) * P],
                                         h2[kt][:, sl],
                                         start=(kt == 0), stop=(kt == NKT - 1))
                    j = d0 // P + dt_
                    nc.scalar.activation(f[dt_][:, sl], ps[:, :], AF.Gelu,
                                         bias=c_fb1[:, j:j + 1])
                # ff2 partial, accumulated across halves in fp32 SBUF
                for m in range(NKT):
                    ps = ps2.tile([P, 512], F32,
                                  name=f"f2ps_{half}_{m}_{ch}", tag="f2")
                    for kt in range(HKT):
                        nc.tensor.matmul(ps[:, :],
                                         w2[kt][:, m * P:(m + 1) * P],
                                         f[kt][:, sl],
                                         start=(kt == 0), stop=(kt == HKT - 1))
                    if half == 0:
                        nc.vector.tensor_scalar(acc[m][:, sl], ps[:, :], 1.0,
                                                None, ALU.mult)
                    else:
                        nc.vector.tensor_tensor(acc[m][:, sl], acc[m][:, sl],
                                                ps[:, :], ALU.add)
                        nc.vector.tensor_tensor(acc[m][:, sl], acc[m][:, sl],
                                                x2[m][:, sl], ALU.add)
                        nc.scalar.activation(acc[m][:, sl], acc[m][:, sl],
                                             AF.Identity,
                                             bias=c_fb2[:, m:m + 1])
                        nc.sync.dma_start(out=outT[m * P:(m + 1) * P, sl],
                                          in_=acc[m][:, sl])
    s_h2.close()
    s_x2.close()
    ctx.close()


_CACHED_NC = None


def _get_nc():
    global _CACHED_NC
    if _CACHED_NC is None:
        nc = bass.Bass("TRN2", num_devices=N_CORES)
        with tile.TileContext(nc) as tc:
            build(tc)
        _CACHED_NC = nc
    return _CACHED_NC


def _bf(a):
    return np.ascontiguousarray(a).astype(ml_dtypes.bfloat16)


def make_in_maps(inputs):
    x = np.asarray(inputs["x"], np.float32)
    qkv_w = np.asarray(inputs["qkv_w"], np.float32)
    proj_w = np.asarray(inputs["proj_w"], np.float32)
    ff_w1 = np.asarray(inputs["ff_w1"], np.float32)
    ff_w2 = np.asarray(inputs["ff_w2"], np.float32)

    def vec_tiles(name, n):
        a = np.asarray(inputs[name], np.float32)
        return np.ascontiguousarray(a.reshape(n, P).T)

    wprojT = _bf(proj_w.T)
    wff1T = _bf(ff_w1.T)
    wff2T = _bf(ff_w2.T)
    ln1w = vec_tiles("ln1_w", NKT)
    ln1b = vec_tiles("ln1_b", NKT)
    ln2w = vec_tiles("ln2_w", NKT)
    ln2b = vec_tiles("ln2_b", NKT)
    fb1 = vec_tiles("ff_b1", DFF // P)
    fb2 = vec_tiles("ff_b2", NKT)

    in_maps = []
    for c in range(N_CORES):
        b, g = c // 2, c % 2
        wq = qkv_w[g * 512:(g + 1) * 512, :]
        wk = qkv_w[C + g * 512: C + (g + 1) * 512, :]
        wv = qkv_w[2 * C + g * 512: 2 * C + (g + 1) * 512, :]
        wqkvT = _bf(np.concatenate([wq.T, wk.T, wv.T], axis=1))
        mabv = np.zeros((64, 2), np.float32)
        mabv[:, g] = 1.0
        in_maps.append({
            "xT": _bf(x[b].T),
            "xmT": np.ascontiguousarray(x[b, g * MYT:(g + 1) * MYT, :].T),
            "wqkvT": wqkvT,
            "wprojT": wprojT,
            "wff1T": wff1T,
            "wff2T": wff2T,
            "ln1w": ln1w, "ln1b": ln1b, "ln2w": ln2w, "ln2b": ln2b,
            "fb1": fb1, "fb2": fb2, "mab": mabv,
        })
    return in_maps


LAST_RESULT = None


def kernel(**inputs):
    global LAST_RESULT
    nc = _get_nc()
    in_maps = make_in_maps(inputs)
    res = run_bass_kernel_spmd(
        nc, in_maps, core_ids=list(range(N_CORES)),
        trace=bool(int(os.environ.get("KERNEL_TRACE", "0"))))
    LAST_RESULT = res
    if res.exec_time_ns is not None:
        print(f"HW exec time: {res.exec_time_ns} ns")
    out = np.zeros((B, T, C), np.float32)
    for c in range(N_CORES):
        b, g = c // 2, c % 2
        out[b, g * MYT:(g + 1) * MYT, :] = res.results[c]["outT"].T
    return (out, np.zeros((), np.float32))


if __name__ == "__main__":
    _get_nc()
    print("built ok")
